# revision 15
# baseline (speedup 1.0000x reference)
# Causal self-attention (single head, full dim) on 8 NeuronCores.
#
# Problem: x (4, 2048, 1024) f32; Wq/Wk/Wv (1024, 1024) f32.
#   out = softmax(causal((x Wq)(x Wk)^T) / 32) @ (x Wv)
#
# Algebraic restructuring (exploits single-head full-dim, no nonlinearity
# between the projections and the attention products):
#   scores  = x  (Wq Wk^T) x^T  -> G = x_q W_qk (q-rows only), S = G x^T
#   context = (P x) Wv           -> PX = P x (contract keys), then @ Wv
# This removes the K and V projections entirely (the only key-proportional
# work left is S and PX, which is inherent), so nothing is duplicated
# across the 2 cores that share a batch and no collectives are needed.
# W_qk = Wq @ Wk^T is precomputed on the host in fp64.
#
# Sharding: 2 cores per batch element. Core (b, h) handles query row-tiles
# g = 2j + h (j = 0..7, 128 rows each) of batch b. Slot j uses kv-length
# 256*(j+1), which exactly covers causality for both pair members and makes
# every core's static program identical (SPMD) and exactly load-balanced.
# Causal masking only differs in the last 256 key columns of each slot,
# supplied as a tiny additive-mask input that depends only on h.
#
# Per-core pipeline (bf16 matmuls, fp32 PSUM):
#   GT = W_qk^T x_q^T            [d, q]     128 MMs
#   per slot j (descending):
#     S_j  = GT_j^T XT[:, :L]    PSUM f32   scores
#     P_j  = exp(S_j/32 + mask)  ACT, row-sum via accum_out (no max pass:
#                                |S|/32 <~ 6 so exp is fp32-safe)
#     PT_j = PE-transpose(P_j)
#     PX_j = PT_j^T X_nat        [q, d]
#     PXT_j = PE-transpose(PX_j)
#     C_j  = (PXT_j^T Wv) / rowsum
import numpy as np
import ml_dtypes

B, S, D, PD = 4, 2048, 1024, 1024
PP = 128              # partitions
DK = D // PP          # 8 contraction subtiles
NQT = 8               # q tiles per core
NKT = S // PP         # 16 kv tiles
NEG = -1.0e9
SCALE = 1.0 / 32.0    # 1/sqrt(PD)

_CACHE = {}


def _build_nc():
    import concourse.mybir as mybir
    import concourse.tile as tile
    from concourse import bacc

    bf16 = mybir.dt.bfloat16
    f32 = mybir.dt.float32
    nc = bacc.Bacc("TRN2", target_bir_lowering=False)

    xt_q_d = nc.dram_tensor("xt_q", (D, PP * NQT), bf16, kind="ExternalInput")
    xt_kv_d = nc.dram_tensor("xt_kv", (D, S), bf16, kind="ExternalInput")
    x_nat_d = nc.dram_tensor("x_nat", (S, D), bf16, kind="ExternalInput")
    wqk_d = nc.dram_tensor("wqk", (D, PD), bf16, kind="ExternalInput")
    wv_d = nc.dram_tensor("wv", (D, PD), bf16, kind="ExternalInput")
    mask_d = nc.dram_tensor("mask", (PP, 256), f32, kind="ExternalInput")
    ident_d = nc.dram_tensor("ident", (PP, PP), bf16, kind="ExternalInput")
    out_d = nc.dram_tensor("out", (PP * NQT, PD), f32, kind="ExternalOutput")

    with tile.TileContext(nc) as tc:
        with (
            tc.tile_pool(name="persist", bufs=1) as persist,
            tc.tile_pool(name="ppool", bufs=2) as ppool,
            tc.tile_pool(name="ptpool", bufs=1) as ptpool,
            tc.tile_pool(name="pxpool", bufs=2) as pxpool,
            tc.tile_pool(name="pxtpool", bufs=2) as pxtpool,
            tc.tile_pool(name="outpool", bufs=2) as outpool,
            tc.tile_pool(name="stats", bufs=4) as stats,
            tc.tile_pool(name="spsum", bufs=3, space="PSUM") as spsum,
            tc.tile_pool(name="ptpsum", bufs=2, space="PSUM") as ptpsum,
            tc.tile_pool(name="cpsum", bufs=3, space="PSUM") as cpsum,
        ):
            ident = persist.tile([PP, PP], bf16, tag="ident")
            nc.sync.dma_start(out=ident, in_=ident_d[:, :])
            # Small PE warmup sized to the wqk-load wait: keeps TensorE busy
            # (and trips the HAM clock-gate to full rate) during the window
            # between the identity landing and the first weight slices.
            for _ in range(12):
                wps = ptpsum.tile([PP, PP], bf16, tag="ptps")
                nc.tensor.transpose(wps, ident, ident)
            mask_sb = persist.tile([PP, 256], f32, tag="mask")
            nc.sync.dma_start(out=mask_sb, in_=mask_d[:, :])

            xt_q = persist.tile([PP, DK, PP * NQT], bf16, tag="xt_q")
            xt_kv = persist.tile([PP, DK, S], bf16, tag="xt_kv")
            x_nat = persist.tile([PP, NKT, D], bf16, tag="x_nat")
            wqk_sb = persist.tile([PP, DK, PD], bf16, tag="wqk")
            wv_sb = persist.tile([PP, DK, PD], bf16, tag="wv")
            gt = persist.tile([PP, DK, PP * NQT], bf16, tag="gt")

            # Loads in phase-need order: GT proj inputs, then score inputs,
            # then PX/ctx inputs.
            for ki in range(DK):
                nc.sync.dma_start(
                    out=wqk_sb[:, ki, :512],
                    in_=wqk_d[ki * PP : (ki + 1) * PP, :512],
                )
                nc.sync.dma_start(
                    out=xt_q[:, ki, :], in_=xt_q_d[ki * PP : (ki + 1) * PP, :]
                )
            for ki in range(DK):
                nc.sync.dma_start(
                    out=wqk_sb[:, ki, 512:],
                    in_=wqk_d[ki * PP : (ki + 1) * PP, 512:],
                )
            for ki in range(DK):
                nc.sync.dma_start(
                    out=xt_kv[:, ki, :], in_=xt_kv_d[ki * PP : (ki + 1) * PP, :]
                )
            for ko in range(NKT):
                nc.sync.dma_start(
                    out=x_nat[:, ko, :], in_=x_nat_d[ko * PP : (ko + 1) * PP, :]
                )
            for ki in range(DK):
                nc.sync.dma_start(
                    out=wv_sb[:, ki, :], in_=wv_d[ki * PP : (ki + 1) * PP, :]
                )

            # ---- Phase 1: GT[d, q] = W_qk^T @ x_q^T ----
            for do in range(DK):
                for qc in range(2):
                    ps = spsum.tile([PP, 512], f32, tag="s")
                    for ki in range(DK):
                        nc.tensor.matmul(
                            ps,
                            lhsT=wqk_sb[:, ki, do * PP : (do + 1) * PP],
                            rhs=xt_q[:, ki, qc * 512 : (qc + 1) * 512],
                            start=(ki == 0),
                            stop=(ki == DK - 1),
                        )
                    nc.vector.tensor_copy(
                        out=gt[:, do, qc * 512 : (qc + 1) * 512], in_=ps
                    )

            # ---- Phase 2: attention slots (software-pipelined scores) ----
            s_tiles = {}

            def emit_scores(j):
                L = 256 * (j + 1)
                nch = (L + 511) // 512
                tiles = []
                for c in range(nch):
                    w = min(512, L - 512 * c)
                    ps = spsum.tile([PP, 512], f32, tag="s")
                    for ki in range(DK):
                        nc.tensor.matmul(
                            ps[:, :w],
                            lhsT=gt[:, ki, j * PP : (j + 1) * PP],
                            rhs=xt_kv[:, ki, c * 512 : c * 512 + w],
                            start=(ki == 0),
                            stop=(ki == DK - 1),
                        )
                    tiles.append((ps, w))
                # softmax stage emitted with the scores so its DVE/ACT ops
                # sit ahead of the copy traffic of the previous slot's rest
                # stage in those engines' queues (shorter critical chain).
                off = (L - 256) - 512 * (nch - 1)
                last_ps, _wlast = tiles[-1]
                nc.vector.tensor_add(
                    out=last_ps[:, off : off + 256],
                    in0=last_ps[:, off : off + 256],
                    in1=mask_sb,
                )
                p_t = ppool.tile([PP, S], bf16, tag="p")
                rs = stats.tile([PP, 4], f32, tag="rs")
                for c, (ps, w) in enumerate(tiles):
                    nc.scalar.activation(
                        out=p_t[:, c * 512 : c * 512 + w],
                        in_=ps[:, :w],
                        func=mybir.ActivationFunctionType.Exp,
                        scale=SCALE,
                        accum_out=rs[:, c : c + 1],
                    )
                r = stats.tile([PP, 1], f32, tag="r")
                nc.vector.reduce_sum(r, rs[:, :nch], axis=mybir.AxisListType.X)
                rinv = stats.tile([PP, 1], f32, tag="rinv")
                nc.vector.reciprocal(rinv, r)
                s_tiles[j] = (p_t, rinv)

            def emit_rest(j):
                nkv = 2 * (j + 1)
                p_t, rinv = s_tiles.pop(j)

                # PT[k, q] tiles via PE transpose
                pt_t = ptpool.tile([PP, NKT, PP], bf16, tag="pt")
                for t in range(nkv):
                    pps = ptpsum.tile([PP, PP], bf16, tag="ptps")
                    nc.tensor.transpose(pps, p_t[:, t * PP : (t + 1) * PP], ident)
                    nc.vector.tensor_copy(out=pt_t[:, t, :], in_=pps)

                pxt_t = pxtpool.tile([PP, DK, PP], bf16, tag="pxt")
                if nkv <= 4:
                    # Small slots: compute PXT[d, q] directly (N=128 matmuls
                    # accumulating over the few kv tiles) — the PX -> cast ->
                    # transpose -> copy chain latency would exceed the extra
                    # PE cost here.
                    for dd in range(DK):
                        cps = cpsum.tile([PP, 512], f32, tag="c")
                        for t in range(nkv):
                            nc.tensor.matmul(
                                cps[:, :PP],
                                lhsT=x_nat[:, t, dd * PP : (dd + 1) * PP],
                                rhs=pt_t[:, t, :],
                                start=(t == 0),
                                stop=(t == nkv - 1),
                            )
                        nc.vector.tensor_copy(out=pxt_t[:, dd, :], in_=cps[:, :PP])
                else:
                    # PX[q, d] = P @ X
                    px_t = pxpool.tile([PP, D], bf16, tag="px")
                    for dc in range(2):
                        cps = cpsum.tile([PP, 512], f32, tag="c")
                        for t in range(nkv):
                            nc.tensor.matmul(
                                cps,
                                lhsT=pt_t[:, t, :],
                                rhs=x_nat[:, t, dc * 512 : (dc + 1) * 512],
                                start=(t == 0),
                                stop=(t == nkv - 1),
                            )
                        for q4 in range(4):
                            nc.vector.tensor_copy(
                                out=px_t[:, dc * 512 + q4 * PP : dc * 512 + (q4 + 1) * PP],
                                in_=cps[:, q4 * PP : (q4 + 1) * PP],
                            )

                    # PXT[d, q] via PE transpose
                    for dd in range(DK):
                        pps = ptpsum.tile([PP, PP], bf16, tag="ptps")
                        nc.tensor.transpose(
                            pps, px_t[:, dd * PP : (dd + 1) * PP], ident
                        )
                        nc.vector.tensor_copy(out=pxt_t[:, dd, :], in_=pps)

                # C[q, dv] = (PX @ Wv) / rowsum
                out_t = outpool.tile([PP, PD], f32, tag="out")
                for vc in range(2):
                    cps = cpsum.tile([PP, 512], f32, tag="c")
                    for ki in range(DK):
                        nc.tensor.matmul(
                            cps,
                            lhsT=pxt_t[:, ki, :],
                            rhs=wv_sb[:, ki, vc * 512 : (vc + 1) * 512],
                            start=(ki == 0),
                            stop=(ki == DK - 1),
                        )
                    nc.vector.tensor_scalar_mul(
                        out=out_t[:, vc * 512 : (vc + 1) * 512],
                        in0=cps,
                        scalar1=rinv,
                    )
                nc.sync.dma_start(out=out_d[j * PP : (j + 1) * PP, :], in_=out_t)

            # Descending slot order: the tail slot's softmax/PX/ctx chain is
            # then the smallest (256-wide) one.
            order = list(range(NQT - 1, -1, -1))
            emit_scores(order[0])
            for idx, j in enumerate(order):
                if idx + 1 < len(order):
                    emit_scores(order[idx + 1])
                emit_rest(j)

    nc.finalize()
    return nc


def _get_nc():
    if "nc" not in _CACHE:
        _CACHE["nc"] = _build_nc()
    return _CACHE["nc"]


def _prep_inputs(x, Wq, Wk, Wv):
    bf = ml_dtypes.bfloat16
    wqk = (Wq.astype(np.float64) @ Wk.astype(np.float64).T).astype(bf)
    wqk = np.ascontiguousarray(wqk)
    wv_b = np.ascontiguousarray(Wv.astype(bf))
    ri = np.arange(PP)[:, None]
    ci = np.arange(256)[None, :]
    masks = [
        np.where(ci <= ri + PP * h, 0.0, NEG).astype(np.float32) for h in (0, 1)
    ]
    in_maps = []
    for b in range(B):
        xb = x[b].astype(bf)
        xt = np.ascontiguousarray(xb.T)  # (D, S)
        for h in (0, 1):
            qrows = np.concatenate(
                [np.arange(PP * (2 * j + h), PP * (2 * j + h) + PP) for j in range(NQT)]
            )
            xq = np.ascontiguousarray(xb[qrows].T)  # (D, 1024)
            in_maps.append(
                {
                    "xt_q": xq,
                    "xt_kv": xt,
                    "x_nat": xb,
                    "wqk": wqk,
                    "wv": wv_b,
                    "mask": masks[h],
                    "ident": np.eye(PP, dtype=bf),
                }
            )
    return in_maps


def kernel(x, Wq, Wk, Wv, _trace=False):
    from concourse.bass_utils import run_bass_kernel_spmd

    x = np.asarray(x, dtype=np.float32)
    Wq = np.asarray(Wq, dtype=np.float32)
    Wk = np.asarray(Wk, dtype=np.float32)
    Wv = np.asarray(Wv, dtype=np.float32)

    nc = _get_nc()
    in_maps = _prep_inputs(x, Wq, Wk, Wv)
    kwargs = {}
    if _trace:
        kwargs = {"trace": True, "trace_cores": [0]}
    res = run_bass_kernel_spmd(nc, in_maps, core_ids=list(range(8)), **kwargs)
    _CACHE["last_result"] = res

    out = np.empty((B, S, PD), dtype=np.float32)
    for b in range(B):
        for h in (0, 1):
            o = res.results[b * 2 + h]["out"]
            for j in range(NQT):
                g = 2 * j + h
                out[b, PP * g : PP * (g + 1)] = o[PP * j : PP * (j + 1)]
    return out


# revision 16
# speedup vs baseline: 1.0280x; 1.0280x over previous
# Causal self-attention (single head, full dim) on 8 NeuronCores.
#
# Problem: x (4, 2048, 1024) f32; Wq/Wk/Wv (1024, 1024) f32.
#   out = softmax(causal((x Wq)(x Wk)^T) / 32) @ (x Wv)
#
# Algebraic restructuring (exploits single-head full-dim, no nonlinearity
# between the projections and the attention products):
#   scores  = x  (Wq Wk^T) x^T  -> G = x_q W_qk (q-rows only), S = G x^T
#   context = (P x) Wv           -> PX = P x (contract keys), then @ Wv
# This removes the K and V projections entirely (the only key-proportional
# work left is S and PX, which is inherent), so nothing is duplicated
# across the 2 cores that share a batch and no collectives are needed.
# W_qk = Wq @ Wk^T is precomputed on the host in fp64.
#
# Sharding: 2 cores per batch element. Core (b, h) handles query row-tiles
# g = 2j + h (j = 0..7, 128 rows each) of batch b. Slot j uses kv-length
# 256*(j+1), which exactly covers causality for both pair members and makes
# every core's static program identical (SPMD) and exactly load-balanced.
# Causal masking only differs in the last 256 key columns of each slot,
# supplied as a tiny additive-mask input that depends only on h.
#
# Per-core pipeline (bf16 matmuls, fp32 PSUM):
#   GT = W_qk^T x_q^T            [d, q]     128 MMs
#   per slot j (descending):
#     S_j  = GT_j^T XT[:, :L]    PSUM f32   scores
#     P_j  = exp(S_j/32 + mask)  ACT, row-sum via accum_out (no max pass:
#                                |S|/32 <~ 6 so exp is fp32-safe)
#     PT_j = PE-transpose(P_j)
#     PX_j = PT_j^T X_nat        [q, d]
#     PXT_j = PE-transpose(PX_j)
#     C_j  = (PXT_j^T Wv) / rowsum
import numpy as np
import ml_dtypes

B, S, D, PD = 4, 2048, 1024, 1024
PP = 128              # partitions
DK = D // PP          # 8 contraction subtiles
NQT = 8               # q tiles per core
NKT = S // PP         # 16 kv tiles
NEG = -1.0e9
SCALE = 1.0 / 32.0    # 1/sqrt(PD)

_CACHE = {}


def _build_nc():
    import concourse.mybir as mybir
    import concourse.tile as tile
    from concourse import bacc

    bf16 = mybir.dt.bfloat16
    f32 = mybir.dt.float32
    nc = bacc.Bacc("TRN2", target_bir_lowering=False)

    xt_q_d = nc.dram_tensor("xt_q", (D, PP * NQT), bf16, kind="ExternalInput")
    xt_kv_d = nc.dram_tensor("xt_kv", (D, S), bf16, kind="ExternalInput")
    x_nat_d = nc.dram_tensor("x_nat", (S, D), bf16, kind="ExternalInput")
    wqk_d = nc.dram_tensor("wqk", (D, PD), bf16, kind="ExternalInput")
    wv_d = nc.dram_tensor("wv", (D, PD), bf16, kind="ExternalInput")
    mask_d = nc.dram_tensor("mask", (PP, 256), f32, kind="ExternalInput")
    ident_d = nc.dram_tensor("ident", (PP, PP), bf16, kind="ExternalInput")
    out_d = nc.dram_tensor("out", (PP * NQT, PD), f32, kind="ExternalOutput")

    with tile.TileContext(nc) as tc:
        with (
            tc.tile_pool(name="persist", bufs=1) as persist,
            tc.tile_pool(name="ppool", bufs=2) as ppool,
            tc.tile_pool(name="ptpool", bufs=1) as ptpool,
            tc.tile_pool(name="pxpool", bufs=2) as pxpool,
            tc.tile_pool(name="pxtpool", bufs=2) as pxtpool,
            tc.tile_pool(name="outpool", bufs=2) as outpool,
            tc.tile_pool(name="stats", bufs=4) as stats,
            tc.tile_pool(name="spsum", bufs=3, space="PSUM") as spsum,
            tc.tile_pool(name="ptpsum", bufs=2, space="PSUM") as ptpsum,
            tc.tile_pool(name="cpsum", bufs=3, space="PSUM") as cpsum,
        ):
            ident = persist.tile([PP, PP], bf16, tag="ident")
            nc.sync.dma_start(out=ident, in_=ident_d[:, :])
            # Small PE warmup sized to the wqk-load wait: keeps TensorE busy
            # (and trips the HAM clock-gate to full rate) during the window
            # between the identity landing and the first weight slices.
            for _ in range(12):
                wps = ptpsum.tile([PP, PP], bf16, tag="ptps")
                nc.tensor.transpose(wps, ident, ident)
            mask_sb = persist.tile([PP, 256], f32, tag="mask")
            nc.sync.dma_start(out=mask_sb, in_=mask_d[:, :])

            xt_q = persist.tile([PP, DK, PP * NQT], bf16, tag="xt_q")
            xt_kv = persist.tile([PP, DK, S], bf16, tag="xt_kv")
            x_nat = persist.tile([PP, NKT, D], bf16, tag="x_nat")
            wqk_sb = persist.tile([PP, DK, PD], bf16, tag="wqk")
            wv_sb = persist.tile([PP, DK, PD], bf16, tag="wv")
            gt = persist.tile([PP, DK, PP * NQT], bf16, tag="gt")

            # Loads in phase-need order: GT proj inputs, then score inputs,
            # then PX/ctx inputs.
            for ki in range(DK):
                nc.sync.dma_start(
                    out=wqk_sb[:, ki, :512],
                    in_=wqk_d[ki * PP : (ki + 1) * PP, :512],
                )
                nc.sync.dma_start(
                    out=xt_q[:, ki, :], in_=xt_q_d[ki * PP : (ki + 1) * PP, :]
                )
            for ki in range(DK):
                nc.sync.dma_start(
                    out=wqk_sb[:, ki, 512:],
                    in_=wqk_d[ki * PP : (ki + 1) * PP, 512:],
                )
            for ki in range(DK):
                nc.sync.dma_start(
                    out=xt_kv[:, ki, :], in_=xt_kv_d[ki * PP : (ki + 1) * PP, :]
                )
            for ko in range(NKT):
                nc.sync.dma_start(
                    out=x_nat[:, ko, :], in_=x_nat_d[ko * PP : (ko + 1) * PP, :]
                )
            for ki in range(DK):
                nc.sync.dma_start(
                    out=wv_sb[:, ki, :], in_=wv_d[ki * PP : (ki + 1) * PP, :]
                )

            # ---- Phase 1: GT[d, q] = W_qk^T @ x_q^T ----
            for do in range(DK):
                for qc in range(2):
                    ps = spsum.tile([PP, 512], f32, tag="s")
                    for ki in range(DK):
                        nc.tensor.matmul(
                            ps,
                            lhsT=wqk_sb[:, ki, do * PP : (do + 1) * PP],
                            rhs=xt_q[:, ki, qc * 512 : (qc + 1) * 512],
                            start=(ki == 0),
                            stop=(ki == DK - 1),
                        )
                    nc.vector.tensor_copy(
                        out=gt[:, do, qc * 512 : (qc + 1) * 512], in_=ps
                    )

            # ---- Phase 2: attention slots (software-pipelined scores) ----
            s_tiles = {}

            def emit_scores(j):
                L = 256 * (j + 1)
                nch = (L + 511) // 512
                tiles = []
                for c in range(nch):
                    w = min(512, L - 512 * c)
                    ps = spsum.tile([PP, 512], f32, tag="s")
                    for ki in range(DK):
                        nc.tensor.matmul(
                            ps[:, :w],
                            lhsT=gt[:, ki, j * PP : (j + 1) * PP],
                            rhs=xt_kv[:, ki, c * 512 : c * 512 + w],
                            start=(ki == 0),
                            stop=(ki == DK - 1),
                        )
                    tiles.append((ps, w))
                # softmax stage emitted with the scores so its DVE/ACT ops
                # sit ahead of the copy traffic of the previous slot's rest
                # stage in those engines' queues (shorter critical chain).
                off = (L - 256) - 512 * (nch - 1)
                last_ps, _wlast = tiles[-1]
                nc.vector.tensor_add(
                    out=last_ps[:, off : off + 256],
                    in0=last_ps[:, off : off + 256],
                    in1=mask_sb,
                )
                p_t = ppool.tile([PP, S], bf16, tag="p")
                rs = stats.tile([PP, 4], f32, tag="rs")
                for c, (ps, w) in enumerate(tiles):
                    nc.scalar.activation(
                        out=p_t[:, c * 512 : c * 512 + w],
                        in_=ps[:, :w],
                        func=mybir.ActivationFunctionType.Exp,
                        scale=SCALE,
                        accum_out=rs[:, c : c + 1],
                    )
                r = stats.tile([PP, 1], f32, tag="r")
                nc.vector.reduce_sum(r, rs[:, :nch], axis=mybir.AxisListType.X)
                rinv = stats.tile([PP, 1], f32, tag="rinv")
                nc.vector.reciprocal(rinv, r)
                s_tiles[j] = (p_t, rinv)

            def emit_rest(j):
                nkv = 2 * (j + 1)
                p_t, rinv = s_tiles.pop(j)

                # PT[k, q] tiles via PE transpose
                pt_t = ptpool.tile([PP, NKT, PP], bf16, tag="pt")
                for t in range(nkv):
                    pps = ptpsum.tile([PP, PP], bf16, tag="ptps")
                    nc.tensor.transpose(pps, p_t[:, t * PP : (t + 1) * PP], ident)
                    nc.vector.tensor_copy(out=pt_t[:, t, :], in_=pps)

                pxt_t = pxtpool.tile([PP, DK, PP], bf16, tag="pxt")
                if nkv <= 6:
                    # Small slots: compute PXT[d, q] directly (N=128 matmuls
                    # accumulating over the few kv tiles) — the PX -> cast ->
                    # transpose -> copy chain latency would exceed the extra
                    # PE cost here.
                    for dd in range(DK):
                        cps = cpsum.tile([PP, 512], f32, tag="c")
                        for t in range(nkv):
                            nc.tensor.matmul(
                                cps[:, :PP],
                                lhsT=x_nat[:, t, dd * PP : (dd + 1) * PP],
                                rhs=pt_t[:, t, :],
                                start=(t == 0),
                                stop=(t == nkv - 1),
                            )
                        nc.vector.tensor_copy(out=pxt_t[:, dd, :], in_=cps[:, :PP])
                else:
                    # PX[q, d] = P @ X
                    px_t = pxpool.tile([PP, D], bf16, tag="px")
                    for dc in range(2):
                        cps = cpsum.tile([PP, 512], f32, tag="c")
                        for t in range(nkv):
                            nc.tensor.matmul(
                                cps,
                                lhsT=pt_t[:, t, :],
                                rhs=x_nat[:, t, dc * 512 : (dc + 1) * 512],
                                start=(t == 0),
                                stop=(t == nkv - 1),
                            )
                        for q4 in range(4):
                            nc.vector.tensor_copy(
                                out=px_t[:, dc * 512 + q4 * PP : dc * 512 + (q4 + 1) * PP],
                                in_=cps[:, q4 * PP : (q4 + 1) * PP],
                            )

                    # PXT[d, q] via PE transpose
                    for dd in range(DK):
                        pps = ptpsum.tile([PP, PP], bf16, tag="ptps")
                        nc.tensor.transpose(
                            pps, px_t[:, dd * PP : (dd + 1) * PP], ident
                        )
                        nc.vector.tensor_copy(out=pxt_t[:, dd, :], in_=pps)

                # C[q, dv] = (PX @ Wv) / rowsum
                out_t = outpool.tile([PP, PD], f32, tag="out")
                for vc in range(2):
                    cps = cpsum.tile([PP, 512], f32, tag="c")
                    for ki in range(DK):
                        nc.tensor.matmul(
                            cps,
                            lhsT=pxt_t[:, ki, :],
                            rhs=wv_sb[:, ki, vc * 512 : (vc + 1) * 512],
                            start=(ki == 0),
                            stop=(ki == DK - 1),
                        )
                    nc.scalar.mul(
                        out=out_t[:, vc * 512 : (vc + 1) * 512],
                        in_=cps,
                        mul=rinv,
                    )
                nc.sync.dma_start(out=out_d[j * PP : (j + 1) * PP, :], in_=out_t)

            # Descending slot order: the tail slot's softmax/PX/ctx chain is
            # then the smallest (256-wide) one.
            order = list(range(NQT - 1, -1, -1))
            emit_scores(order[0])
            for idx, j in enumerate(order):
                if idx + 1 < len(order):
                    emit_scores(order[idx + 1])
                emit_rest(j)

    nc.finalize()
    return nc


def _get_nc():
    if "nc" not in _CACHE:
        _CACHE["nc"] = _build_nc()
    return _CACHE["nc"]


def _prep_inputs(x, Wq, Wk, Wv):
    bf = ml_dtypes.bfloat16
    wqk = (Wq.astype(np.float64) @ Wk.astype(np.float64).T).astype(bf)
    wqk = np.ascontiguousarray(wqk)
    wv_b = np.ascontiguousarray(Wv.astype(bf))
    ri = np.arange(PP)[:, None]
    ci = np.arange(256)[None, :]
    masks = [
        np.where(ci <= ri + PP * h, 0.0, NEG).astype(np.float32) for h in (0, 1)
    ]
    in_maps = []
    for b in range(B):
        xb = x[b].astype(bf)
        xt = np.ascontiguousarray(xb.T)  # (D, S)
        for h in (0, 1):
            qrows = np.concatenate(
                [np.arange(PP * (2 * j + h), PP * (2 * j + h) + PP) for j in range(NQT)]
            )
            xq = np.ascontiguousarray(xb[qrows].T)  # (D, 1024)
            in_maps.append(
                {
                    "xt_q": xq,
                    "xt_kv": xt,
                    "x_nat": xb,
                    "wqk": wqk,
                    "wv": wv_b,
                    "mask": masks[h],
                    "ident": np.eye(PP, dtype=bf),
                }
            )
    return in_maps


def kernel(x, Wq, Wk, Wv, _trace=False):
    from concourse.bass_utils import run_bass_kernel_spmd

    x = np.asarray(x, dtype=np.float32)
    Wq = np.asarray(Wq, dtype=np.float32)
    Wk = np.asarray(Wk, dtype=np.float32)
    Wv = np.asarray(Wv, dtype=np.float32)

    nc = _get_nc()
    in_maps = _prep_inputs(x, Wq, Wk, Wv)
    kwargs = {}
    if _trace:
        kwargs = {"trace": True, "trace_cores": [0]}
    res = run_bass_kernel_spmd(nc, in_maps, core_ids=list(range(8)), **kwargs)
    _CACHE["last_result"] = res

    out = np.empty((B, S, PD), dtype=np.float32)
    for b in range(B):
        for h in (0, 1):
            o = res.results[b * 2 + h]["out"]
            for j in range(NQT):
                g = 2 * j + h
                out[b, PP * g : PP * (g + 1)] = o[PP * j : PP * (j + 1)]
    return out
